# revision 4
# baseline (speedup 1.0000x reference)
"""GroupSort(2) Trainium2 Bass kernel.

The reference module
    diff = relu(w_diff @ x);  out = x + w_expand @ diff
with the fixed pair-difference weights is algebraically a pairwise sort:
    out[2k]   = min(x[2k], x[2k+1])
    out[2k+1] = max(x[2k], x[2k+1])
so the kernel is pure elementwise min/max — no matmuls.

Sharding: pure data parallel, batch 32 -> 8 cores x 4.
Per-core layout: x shard [4, 256, 64, 64] viewed as [4, 128, 2, 4096]
(channel pair k lives on partition k; even/odd members side by side).
"""

import numpy as np

import bass_rust
import concourse.mybir as mybir
from concourse.bass import Bass
from concourse.tile import TileContext
from concourse.bass_utils import run_bass_kernel_spmd

N_CORES = 8
B, C, H, W = 32, 256, 64, 64
BS = B // N_CORES          # batches per core
P = 128                    # channel pairs -> SBUF partitions
HW = H * W                 # 4096
K = 2048                   # hw chunk per tile
DT = mybir.dt.float32

_nc_cache = None


def _build():
    global _nc_cache
    if _nc_cache is not None:
        return _nc_cache
    nc = Bass()
    x = nc.declare_dram_parameter("x", [BS, P, 2, HW], DT, isOutput=False)
    out = nc.declare_dram_parameter("out", [BS, P, 2, HW], DT, isOutput=True)
    with TileContext(nc) as tc:
        with (
            tc.tile_pool(name="pin", bufs=3) as pin,
            tc.tile_pool(name="pout", bufs=3) as pout,
        ):
            for b in range(BS):
                for j in range(0, HW, K):
                    tin = pin.tile([P, 2, K], DT)
                    tout = pout.tile([P, 2, K], DT)
                    nc.sync.dma_start(out=tin, in_=x[b, :, :, j : j + K])
                    nc.vector.tensor_tensor(
                        out=tout[:, 0, :], in0=tin[:, 0, :], in1=tin[:, 1, :],
                        op=mybir.AluOpType.min,
                    )
                    nc.vector.tensor_tensor(
                        out=tout[:, 1, :], in0=tin[:, 0, :], in1=tin[:, 1, :],
                        op=mybir.AluOpType.max,
                    )
                    nc.sync.dma_start(out=out[b, :, :, j : j + K], in_=tout)
    # TRN2 allows at most one sync-wait per instruction; Tile can attach
    # several (load sem + slot-release sem). Split the excess onto
    # InstEventSemaphores or neuronxcc codegen rejects the TensorTensors.
    bass_rust.generate_event_semaphores(nc)
    nc.finalize()
    _nc_cache = nc
    return nc


def _run(x, trace=False, **kwargs):
    nc = _build()
    xs = np.ascontiguousarray(np.asarray(x, dtype=np.float32)).reshape(
        N_CORES, BS, P, 2, HW
    )
    in_maps = [{"x": xs[i]} for i in range(N_CORES)]
    res = run_bass_kernel_spmd(
        nc, in_maps, core_ids=list(range(N_CORES)), trace=trace, **kwargs
    )
    out = np.stack([r["out"] for r in res.results], axis=0).reshape(B, C, H, W)
    return out, res


def kernel(x, **_unused_weights):
    out, _ = _run(x)
    return out
